# revision 16
# baseline (speedup 1.0000x reference)
"""Multi-head attention (B=8, T=2048, C=256, H=4) on 8 NeuronCores.

Data-parallel over batch: core b computes batch element b end-to-end.

Per-core dataflow — everything runs "transposed" so the attention
contraction dims land on SBUF partitions and the big score matrices
never need transposing:

  xT   [C, T]      = PE-transpose of x (cast to bf16 during DMA load)
  qkT  [2C, T]     = w_qk @ xT + b_qk   (q/k for all heads; a head PAIR
                                         occupies the two 64-partition
                                         strips of each 128-row chunk)
  v    [T,KC,H,65] = x @ w_v.T + b_v    (fp8e4, natural layout; a ones
                                         column per head for sumexp;
                                         middle dim = k-chunk pairs for
                                         DoubleRow)
  per (q-tile of 512 outer, head-pair / head inner):
    scoresT[k,q]: per group g, chunks 2g/2g+1 of the SAME head into one
      [128, 1024] PSUM via two K=64 bf16 matmuls
    exp: split across TWO engines —
      ScalarE: native Exp (scale=1/8 fused), output fp8e4 directly
      DVE:     Schraudolph int trick: round(score/ln2 + 55.55) -> int8,
               bitcast to fp8e4 (piecewise-linear exp, ~3% rms — fine
               under softmax normalization which cancels common mode)
    PV: fp8e4 DoubleRow matmuls (contraction 256 = 2 chunks at once,
      2 rows/cycle): o2[65, 512] accumulates over 8 groups; row 64 =
      sum(exp) via the ones column
    deferred normalization: unnormalized out2T copied to yT (ScalarE),
      sumexp rows gathered at 32-partition spacing (GpSimd), one batched
      reciprocal_approx_fast per q-tile, K=1 ones-matmuls broadcast
      1/sumexp into a [128, 512] PSUM, one DVE multiply per head-pair
    proj: out[t-chunk, :] = yT[:, t-chunk].T @ w_pT (+ b_p on GpSimd) —
      yT stationary so the output lands natural [T, C]; straight to DRAM.

Engine balance is the point: exp of 16.7M scores costs ~109us on
ScalarE alone (flat 1.2GHz/col), so ~45% of exp groups go to DVE; PV in
fp8 DoubleRow halves its PE time; fixed-function small ops are spread
to whichever engine is idle (GpSimd picks up sumexp gathers + proj
bias). Scores/PSUM stay f32; q/k stay bf16 (fp8 q/k pushed rel err to
~1.8e-2 in numpy — too close to the 2e-2 gate).
"""

import numpy as np

import concourse.bass as bass
import concourse.tile as tile
from concourse import bacc, mybir
from concourse.bass_utils import run_bass_kernel_spmd
from concourse.masks import make_identity

B, T, C = 8, 2048, 256
H, HD = 4, 64
N_CORES = 8
F32 = mybir.dt.float32
F32R = mybir.dt.float32r
BF16 = mybir.dt.bfloat16
F8 = mybir.dt.float8e4
I8 = mybir.dt.int8

QT = 512                # q-tile (columns of scoresT per inner iteration)
NQT = T // QT           # 4
KC = T // 128           # 16 k-chunks of 128
NG = KC // 2            # 8 groups of 2 chunks (DoubleRow pairs)

SCHRAUDOLPH_MUL = 1.0 / np.log(2.0)   # folds the 1/8 logit scale: (1/8)*(8/ln2)
SCHRAUDOLPH_ADD = 56.0 - 0.45         # fp8e4m3 bias 7*8, tuned correction


def build_nc():
    nc = bacc.Bacc("TRN2", target_bir_lowering=False, debug=False,
                   num_devices=N_CORES)

    x_ap = nc.dram_tensor("x", [T, C], F32, kind="ExternalInput").ap()
    wqk_ap = nc.dram_tensor("w_qkT", [C, 2 * C], F32R, kind="ExternalInput").ap()
    wv_ap = nc.dram_tensor("w_vT", [C, C], F32R, kind="ExternalInput").ap()
    wp_ap = nc.dram_tensor("w_pT", [C, C], F32R, kind="ExternalInput").ap()
    bqk_ap = nc.dram_tensor("b_qk", [4, 128], F32, kind="ExternalInput").ap()
    bv_ap = nc.dram_tensor("b_v", [C], F32, kind="ExternalInput").ap()
    bp_ap = nc.dram_tensor("b_p", [C], F32, kind="ExternalInput").ap()
    out_ap = nc.dram_tensor("out", [T, C], F32, kind="ExternalOutput").ap()
    # DRAM bounce buffer for the 1/sumexp rows: lets a partition-stride-0
    # DMA load broadcast each row across 64 partitions (SBUF sources must
    # have nonzero partition stride; DRAM sources may not).
    rec_dram = nc.dram_tensor("recbuf", [NQT, 4, QT], F32, kind="Internal").ap()

    MULT = mybir.AluOpType.mult
    ADD = mybir.AluOpType.add

    with tile.TileContext(nc) as tc:
        with (
            tc.tile_pool(name="consts", bufs=1) as consts,
            tc.tile_pool(name="xstage", bufs=4) as xstage,
            tc.tile_pool(name="xt", bufs=1) as xtp,
            tc.tile_pool(name="qkt", bufs=1) as qktp,
            tc.tile_pool(name="vsb", bufs=1) as vsbp,
            tc.tile_pool(name="expp", bufs=4) as expp,
            tc.tile_pool(name="yt", bufs=1) as ytp,
            tc.tile_pool(name="ostage", bufs=4) as ostage,
            tc.tile_pool(name="small", bufs=6) as small,
            tc.tile_pool(name="scps", bufs=3, space="PSUM") as scps,
            tc.tile_pool(name="o2ps", bufs=1, space="PSUM") as o2ps,
        ):
            # ---- constants / weights -------------------------------------
            ident = consts.tile([128, 128], BF16, tag="ident")
            make_identity(nc, ident[:])

            ones_f = consts.tile([1, 128], F32, tag="ones_f")
            nc.vector.memset(ones_f[:], 1.0)
            ones_r = consts.tile([1, 128], F32R, tag="ones_r")
            nc.vector.tensor_copy(ones_r[:], ones_f[:])

            w_qk = [consts.tile([128, 2 * C], BF16, tag=f"wqk{c}", name=f"wqk{c}") for c in range(2)]
            for c in range(2):
                nc.gpsimd.dma_start(w_qk[c][:], wqk_ap[128 * c:128 * (c + 1), :])
            w_v = [consts.tile([128, C], BF16, tag=f"wv{c}", name=f"wv{c}") for c in range(2)]
            for c in range(2):
                nc.gpsimd.dma_start(w_v[c][:], wv_ap[128 * c:128 * (c + 1), :])
            w_p = [consts.tile([128, C], BF16, tag=f"wp{c}", name=f"wp{c}") for c in range(2)]
            for c in range(2):
                nc.gpsimd.dma_start(w_p[c][:], wp_ap[128 * c:128 * (c + 1), :])

            b_qk = consts.tile([128, 4], F32, tag="bqk")
            nc.gpsimd.dma_start(b_qk[:], bqk_ap.rearrange("c p -> p c"))
            b_p = consts.tile([128, C], F32, tag="bp")
            bp_bc = bass.AP(tensor=bp_ap.tensor, offset=bp_ap.offset,
                            ap=[[0, 128]] + list(bp_ap.ap))
            nc.gpsimd.dma_start(b_p[:], bp_bc)
            b_v = consts.tile([128, C], F32, tag="bv")
            bv_bc = bass.AP(tensor=bv_ap.tensor, offset=bv_ap.offset,
                            ap=[[0, 128]] + list(bv_ap.ap))
            nc.gpsimd.dma_start(b_v[:], bv_bc)
            # b_p as a single f32r row: moving operand of the K=1 proj-bias
            # matmul (f32r bits == f32 bits; dtype only changes PE behavior)
            b_p_row = consts.tile([1, C], F32R, tag="bpr")
            bpr_src = bass.AP(tensor=bp_ap.tensor, offset=bp_ap.offset,
                              ap=[[0, 1]] + list(bp_ap.ap)).bitcast(F32R)
            nc.gpsimd.dma_start(b_p_row[:], bpr_src)

            # ---- stage A: cast-load x to bf16, PE-transpose to xT --------
            xt = [xtp.tile([128, T], BF16, tag=f"xt{c}", name=f"xt{c}") for c in range(2)]
            x_re = x_ap.rearrange("(b a p) c -> b p a c", b=4, p=128)
            xsbig = [None] * 4
            for b in range(4):
                xsbig[b] = xstage.tile([128, 4, C], BF16, tag="xs", name=f"xs{b}")
                nc.gpsimd.dma_start(xsbig[b][:], x_re[b])
            for tt in range(KC):
                xs = xsbig[tt // 4][:, tt % 4, :]
                for c in range(2):
                    if c == 0:
                        ps = scps.tile([128, 128], BF16, tag="sc", name="tp0")
                    else:
                        ps = o2ps.tile([128, 128], BF16, tag=f"o2{tt % 2}",
                                       name=f"tp{tt % 2}")
                    nc.tensor.transpose(ps[:], xs[:, 128 * c:128 * (c + 1)], ident[:])
                    nc.vector.tensor_copy(xt[c][:, 128 * tt:128 * (tt + 1)], ps[:])

            # ---- stage B: qkT [2C, T] = w_qk.T @ xT + b_qk ---------------
            # m-chunk 0: heads 0,1 q | 1: heads 2,3 q | 2: heads 0,1 k | 3: heads 2,3 k
            # n-outer so attention on q-tile 0 can start after n=0.
            qkt = [qktp.tile([128, T], BF16, tag=f"qkt{m}", name=f"qkt{m}") for m in range(4)]
            def stage_b(n, ms=(0, 2, 1, 3)):
                for m in ms:
                    if m % 2 == 0:
                        ps = scps.tile([128, QT], F32, tag="sc", name="bps0")
                    else:
                        ps = o2ps.tile([128, QT], F32, tag=f"o2{m // 2}",
                                       name=f"bps{m}")
                    for c in range(2):
                        nc.tensor.matmul(
                            ps[:], w_qk[c][:, 128 * m:128 * (m + 1)],
                            xt[c][:, QT * n:QT * (n + 1)],
                            start=(c == 0), stop=(c == 1))
                    nc.scalar.activation(
                        qkt[m][:, QT * n:QT * (n + 1)], ps[:],
                        mybir.ActivationFunctionType.Identity,
                        bias=b_qk[:, m:m + 1])

            stage_b(0, ms=(0, 2))

            # ---- stage C: v [T, KC-pairs, H, 65] fp8 + bias + ones col ---
            # inner width 68: DoubleRow fp8 weights need a multiple of 4
            # (4 fp8 per LDW word). col 64 = ones (sumexp), 65-67 = zeros.
            vsb = vsbp.tile([128, KC, H, HD + 4], F8, tag="v", name="v")
            nc.vector.memset(vsb[:, :, :, HD:HD + 4], 0.0)
            nc.vector.memset(vsb[:, :, :, HD:HD + 1], 1.0)
            for tt in range(KC):
                ps = scps.tile([128, C], F32, tag="sc")
                for c in range(2):
                    nc.tensor.matmul(
                        ps[:], xt[c][:, 128 * tt:128 * (tt + 1)], w_v[c][:],
                        start=(c == 0), stop=(c == 1))
                nc.vector.tensor_add(
                    vsb[:, tt, :, 0:HD],
                    ps[:].rearrange("p (h d) -> p h d", h=H),
                    b_v[:].rearrange("p (h d) -> p h d", h=H))

            stage_b(0, ms=(1, 3))
            for n in range(1, NQT):
                stage_b(n)

            # ---- stage D: attention, qt outer / head-pair, head inner ----
            # yt holds UNNORMALIZED out2T; sumexp rows are collected at
            # 32-partition spacing, normalized in one batched reciprocal per
            # q-tile, broadcast via K=1 matmuls, and multiplied in afterwards.
            yt = [ytp.tile([128, T], BF16, tag=f"yt{hp}", name=f"yt{hp}") for hp in range(2)]
            def norm_proj(qt, se):
                # Deferred per-q-tile epilogue, injected into the NEXT tile's
                # attention stream so its (already-resolved) deps never block
                # the in-order engine queues: reciprocal on DVE, 1/sumexp
                # broadcast via partition-stride-0 SBUF DMA, normalization
                # multiplies on GpSimd, proj on PE with bias as a K=1 matmul.
                rec_f = small.tile([97, QT], F32, tag="rec_f")
                nc.vector.reciprocal_approx_fast(rec_f[:], se[:])
                for habs in range(4):
                    nc.sync.dma_start(rec_dram[qt, habs:habs + 1, :],
                                      rec_f[32 * habs:32 * habs + 1, :])
                for hp in range(2):
                    bcS = small.tile([128, QT], F32, tag=f"bc{hp}")
                    a = rec_dram[qt]
                    src_bc = bass.AP(tensor=a.tensor,
                                     offset=a.offset + 2 * hp * QT,
                                     ap=[[QT, 2], [0, 64], [1, QT]])
                    nc.sync.dma_start(bcS[:], src_bc)
                    ys = yt[hp][:, QT * qt:QT * (qt + 1)]
                    nc.gpsimd.tensor_mul(ys, ys, bcS[:])
                for tt in range(qt * QT // 128, (qt + 1) * QT // 128):
                    ps = scps.tile([128, C], F32, tag="sc", name="pps")
                    for c in range(2):
                        nc.tensor.matmul(
                            ps[:], yt[c][:, 128 * tt:128 * (tt + 1)], w_p[c][:],
                            start=(c == 0), stop=False)
                    nc.tensor.matmul(ps[:], ones_r[0:1, :], b_p_row[0:1, :],
                                     start=False, stop=True)
                    ost = ostage.tile([128, C], F32, tag="ost")
                    nc.vector.tensor_copy(ost[:], ps[:])
                    nc.sync.dma_start(out_ap[128 * tt:128 * (tt + 1), :], ost[:])

            pending = []   # (qt, se) epilogues not yet issued
            for qt in range(NQT):
                se = small.tile([97, QT], F32, tag="se")
                for hp in range(2):
                    qT = qkt[hp]
                    kT = qkt[hp + 2]
                    o2 = [o2ps.tile([HD + 4, QT], F32, tag=f"o2{h}", name=f"o2{h}") for h in range(2)]
                    ex = [[None] * NG for _ in range(2)]
                    def pv(h, g):
                        nc.tensor.matmul(
                            o2[h][:],
                            vsb[:, 2 * g:2 * (g + 1), 2 * hp + h, :],
                            ex[h][g].rearrange("p (j n) -> p j n", j=2),
                            start=(g == 0), stop=(g == NG - 1),
                            perf_mode=mybir.MatmulPerfMode.DoubleRow)
                    for g in range(NG):
                        scs = []
                        for h in range(2):
                            sc = scps.tile([128, 2 * QT], F32, tag="sc")
                            for j in range(2):
                                i = 2 * g + j
                                nc.tensor.matmul(
                                    sc[:, QT * j:QT * (j + 1)],
                                    kT[64 * h:64 * (h + 1), 128 * i:128 * (i + 1)],
                                    qT[64 * h:64 * (h + 1), QT * qt:QT * (qt + 1)],
                                    start=True, stop=True)
                            scs.append(sc)
                        for h in range(2):
                            # exp split: h=0 on ScalarE (native exp -> fp8),
                            # h=1 on DVE (Schraudolph int8 -> fp8 bitcast).
                            if h == 1:
                                exi = expp.tile([128, 2 * QT], I8, tag="exv")
                                nc.vector.tensor_scalar(
                                    exi[:], scs[h][:], SCHRAUDOLPH_MUL,
                                    SCHRAUDOLPH_ADD, MULT, ADD)
                                ex[h][g] = exi[:].bitcast(F8)
                            else:
                                exf = expp.tile([128, 2 * QT], F8, tag="exs")
                                nc.scalar.activation(
                                    exf[:], scs[h][:],
                                    mybir.ActivationFunctionType.Exp,
                                    bias=0.0, scale=float(HD) ** -0.5)
                                ex[h][g] = exf[:]
                        if g > 0:
                            for h in range(2):
                                pv(h, g - 1)
                        if g == 2 and hp == 0 and pending:
                            norm_proj(*pending.pop())
                    for h in range(2):
                        pv(h, NG - 1)
                    for h in range(2):
                        habs = 2 * hp + h
                        nc.scalar.activation(
                            yt[hp][64 * h:64 * (h + 1), QT * qt:QT * (qt + 1)],
                            o2[h][0:HD, :],
                            mybir.ActivationFunctionType.Copy)
                        nc.scalar.activation(
                            se[32 * habs:32 * habs + 1, :],
                            o2[h][HD:HD + 1, :],
                            mybir.ActivationFunctionType.Copy)
                pending.append((qt, se))
            for args in pending:
                norm_proj(*args)
    nc.compile()
    return nc


_NC_CACHE = []


def _get_nc():
    if not _NC_CACHE:
        _NC_CACHE.append(build_nc())
    return _NC_CACHE[0]


def make_in_maps(x, w_qkv, b_qkv, w_proj, b_proj):
    shared = {
        "w_qkT": np.ascontiguousarray(w_qkv[:2 * C].T, dtype=np.float32),
        "w_vT": np.ascontiguousarray(w_qkv[2 * C:].T, dtype=np.float32),
        "w_pT": np.ascontiguousarray(w_proj.T, dtype=np.float32),
        "b_qk": np.ascontiguousarray(b_qkv[:2 * C].reshape(4, 128), dtype=np.float32),
        "b_v": np.ascontiguousarray(b_qkv[2 * C:], dtype=np.float32),
        "b_p": np.ascontiguousarray(b_proj, dtype=np.float32),
    }
    return [dict(shared, x=np.ascontiguousarray(x[b], dtype=np.float32))
            for b in range(B)]


def run(x, w_qkv, b_qkv, w_proj, b_proj, trace=False):
    nc = _get_nc()
    in_maps = make_in_maps(np.asarray(x), np.asarray(w_qkv), np.asarray(b_qkv),
                           np.asarray(w_proj), np.asarray(b_proj))
    res = run_bass_kernel_spmd(nc, in_maps, list(range(N_CORES)), trace=trace)
    out = np.stack([res.results[b]["out"] for b in range(B)])
    return out, res


def kernel(x, w_qkv, b_qkv, w_proj, b_proj):
    out, _ = run(x, w_qkv, b_qkv, w_proj, b_proj, trace=False)
    return out


# revision 20
# speedup vs baseline: 1.3648x; 1.3648x over previous
"""Multi-head attention (B=8, T=2048, C=256, H=4) on 8 NeuronCores.

Data-parallel over batch: core b computes batch element b end-to-end.

Per-core dataflow — everything runs "transposed" so the attention
contraction dims land on SBUF partitions and the big score matrices
never need transposing:

  xT   [C, T]      = PE-transpose of x (cast to bf16 during DMA load)
  qkT  [2C, T]     = w_qk @ xT + b_qk   (q/k for all heads; a head PAIR
                                         occupies the two 64-partition
                                         strips of each 128-row chunk)
  v    [T,KC,H,65] = x @ w_v.T + b_v    (fp8e4, natural layout; a ones
                                         column per head for sumexp;
                                         middle dim = k-chunk pairs for
                                         DoubleRow)
  per (q-tile of 512 outer, head-pair / head inner):
    scoresT[k,q]: per group g, chunks 2g/2g+1 of the SAME head into one
      [128, 1024] PSUM via two K=64 bf16 matmuls
    exp: split across TWO engines —
      ScalarE: native Exp (scale=1/8 fused), output fp8e4 directly
      DVE:     Schraudolph int trick: round(score/ln2 + 55.55) -> int8,
               bitcast to fp8e4 (piecewise-linear exp, ~3% rms — fine
               under softmax normalization which cancels common mode)
    PV: fp8e4 DoubleRow matmuls (contraction 256 = 2 chunks at once,
      2 rows/cycle): o2[65, 512] accumulates over 8 groups; row 64 =
      sum(exp) via the ones column
    deferred normalization: unnormalized out2T copied to yT (ScalarE),
      sumexp rows gathered at 32-partition spacing (GpSimd), one batched
      reciprocal_approx_fast per q-tile, K=1 ones-matmuls broadcast
      1/sumexp into a [128, 512] PSUM, one DVE multiply per head-pair
    proj: out[t-chunk, :] = yT[:, t-chunk].T @ w_pT (+ b_p on GpSimd) —
      yT stationary so the output lands natural [T, C]; straight to DRAM.

Engine balance is the point: exp of 16.7M scores costs ~109us on
ScalarE alone (flat 1.2GHz/col), so ~45% of exp groups go to DVE; PV in
fp8 DoubleRow halves its PE time; fixed-function small ops are spread
to whichever engine is idle (GpSimd picks up sumexp gathers + proj
bias). Scores/PSUM stay f32; q/k stay bf16 (fp8 q/k pushed rel err to
~1.8e-2 in numpy — too close to the 2e-2 gate).
"""

import numpy as np

import concourse.bass as bass
import concourse.tile as tile
from concourse import bacc, mybir
from concourse.bass_utils import run_bass_kernel_spmd
from concourse.masks import make_identity

B, T, C = 8, 2048, 256
H, HD = 4, 64
N_CORES = 8
F32 = mybir.dt.float32
F32R = mybir.dt.float32r
BF16 = mybir.dt.bfloat16
F8 = mybir.dt.float8e4
I8 = mybir.dt.int8

QT = 512                # q-tile (columns of scoresT per inner iteration)
NQT = T // QT           # 4
KC = T // 128           # 16 k-chunks of 128
NG = KC // 2            # 8 groups of 2 chunks (DoubleRow pairs)

SCHRAUDOLPH_MUL = 1.0 / np.log(2.0)   # folds the 1/8 logit scale: (1/8)*(8/ln2)
SCHRAUDOLPH_ADD = 56.0 - 0.45         # fp8e4m3 bias 7*8, tuned correction


def build_nc():
    nc = bacc.Bacc("TRN2", target_bir_lowering=False, debug=False,
                   num_devices=N_CORES)

    x_ap = nc.dram_tensor("x", [T, C], F32, kind="ExternalInput").ap()
    wqk_ap = nc.dram_tensor("w_qkT", [C, 2 * C], F32R, kind="ExternalInput").ap()
    wv_ap = nc.dram_tensor("w_vT", [C, C], F32R, kind="ExternalInput").ap()
    wp_ap = nc.dram_tensor("w_pT", [C, C], F32R, kind="ExternalInput").ap()
    bqk_ap = nc.dram_tensor("b_qk", [4, 128], F32, kind="ExternalInput").ap()
    bv_ap = nc.dram_tensor("b_v", [C], F32, kind="ExternalInput").ap()
    bp_ap = nc.dram_tensor("b_p", [C], F32, kind="ExternalInput").ap()
    out_ap = nc.dram_tensor("out", [T, C], F32, kind="ExternalOutput").ap()

    MULT = mybir.AluOpType.mult
    ADD = mybir.AluOpType.add

    with tile.TileContext(nc) as tc:
        with (
            tc.tile_pool(name="consts", bufs=1) as consts,
            tc.tile_pool(name="xstage", bufs=4) as xstage,
            tc.tile_pool(name="xt", bufs=1) as xtp,
            tc.tile_pool(name="qkt", bufs=1) as qktp,
            tc.tile_pool(name="vsb", bufs=1) as vsbp,
            tc.tile_pool(name="expp", bufs=4) as expp,
            tc.tile_pool(name="yt", bufs=1) as ytp,
            tc.tile_pool(name="ostage", bufs=4) as ostage,
            tc.tile_pool(name="small", bufs=6) as small,
            tc.tile_pool(name="recdr", bufs=2, space="DRAM") as recdr,
            tc.tile_pool(name="scps", bufs=3, space="PSUM") as scps,
            tc.tile_pool(name="o2ps", bufs=1, space="PSUM") as o2ps,
        ):
            # ---- constants / weights -------------------------------------
            ident = consts.tile([128, 128], BF16, tag="ident")
            make_identity(nc, ident[:])

            w_qk = [consts.tile([128, 2 * C], BF16, tag=f"wqk{c}", name=f"wqk{c}") for c in range(2)]
            for c in range(2):
                nc.gpsimd.dma_start(w_qk[c][:], wqk_ap[128 * c:128 * (c + 1), :])
            w_v = [consts.tile([128, C], BF16, tag=f"wv{c}", name=f"wv{c}") for c in range(2)]
            for c in range(2):
                nc.gpsimd.dma_start(w_v[c][:], wv_ap[128 * c:128 * (c + 1), :])
            w_p = [consts.tile([128, C], BF16, tag=f"wp{c}", name=f"wp{c}") for c in range(2)]
            for c in range(2):
                nc.gpsimd.dma_start(w_p[c][:], wp_ap[128 * c:128 * (c + 1), :])

            b_qk = consts.tile([128, 4], F32, tag="bqk")
            nc.gpsimd.dma_start(b_qk[:], bqk_ap.rearrange("c p -> p c"))
            b_p = consts.tile([128, C], F32, tag="bp")
            bp_bc = bass.AP(tensor=bp_ap.tensor, offset=bp_ap.offset,
                            ap=[[0, 128]] + list(bp_ap.ap))
            nc.gpsimd.dma_start(b_p[:], bp_bc)
            b_v = consts.tile([128, C], F32, tag="bv")
            bv_bc = bass.AP(tensor=bv_ap.tensor, offset=bv_ap.offset,
                            ap=[[0, 128]] + list(bv_ap.ap))
            nc.gpsimd.dma_start(b_v[:], bv_bc)

            # ---- stage A: cast-load x to bf16, PE-transpose to xT --------
            xt = [xtp.tile([128, T], BF16, tag=f"xt{c}", name=f"xt{c}") for c in range(2)]
            x_re = x_ap.rearrange("(b a p) c -> b p a c", b=4, p=128)
            xsbig = [None] * 4
            for b in range(4):
                xsbig[b] = xstage.tile([128, 4, C], BF16, tag="xs", name=f"xs{b}")
                nc.gpsimd.dma_start(xsbig[b][:], x_re[b])
            for tt in range(KC):
                xs = xsbig[tt // 4][:, tt % 4, :]
                for c in range(2):
                    if c == 0:
                        ps = scps.tile([128, 128], BF16, tag="sc", name="tp0")
                    else:
                        ps = o2ps.tile([128, 128], BF16, tag=f"o2{tt % 2}",
                                       name=f"tp{tt % 2}")
                    nc.tensor.transpose(ps[:], xs[:, 128 * c:128 * (c + 1)], ident[:])
                    nc.vector.tensor_copy(xt[c][:, 128 * tt:128 * (tt + 1)], ps[:])

            # ---- stage B: qkT [2C, T] = w_qk.T @ xT + b_qk ---------------
            # m-chunk 0: heads 0,1 q | 1: heads 2,3 q | 2: heads 0,1 k | 3: heads 2,3 k
            # n-outer so attention on q-tile 0 can start after n=0.
            qkt = [qktp.tile([128, T], BF16, tag=f"qkt{m}", name=f"qkt{m}") for m in range(4)]
            def stage_b(n, ms=(0, 2, 1, 3)):
                for m in ms:
                    if m % 2 == 0:
                        ps = scps.tile([128, QT], F32, tag="sc", name="bps0")
                    else:
                        ps = o2ps.tile([128, QT], F32, tag=f"o2{m // 2}",
                                       name=f"bps{m}")
                    for c in range(2):
                        nc.tensor.matmul(
                            ps[:], w_qk[c][:, 128 * m:128 * (m + 1)],
                            xt[c][:, QT * n:QT * (n + 1)],
                            start=(c == 0), stop=(c == 1))
                    nc.scalar.activation(
                        qkt[m][:, QT * n:QT * (n + 1)], ps[:],
                        mybir.ActivationFunctionType.Identity,
                        bias=b_qk[:, m:m + 1])

            stage_b(0, ms=(0, 2))

            # ---- stage C: v [T, KC-pairs, H, 65] fp8 + bias + ones col ---
            # inner width 68: DoubleRow fp8 weights need a multiple of 4
            # (4 fp8 per LDW word). col 64 = ones (sumexp), 65-67 = zeros.
            vsb = vsbp.tile([128, KC, H, HD + 4], F8, tag="v", name="v")
            nc.vector.memset(vsb[:, :, :, HD:HD + 4], 0.0)
            nc.vector.memset(vsb[:, :, :, HD:HD + 1], 1.0)
            for tt in range(KC):
                ps = scps.tile([128, C], F32, tag="sc")
                for c in range(2):
                    nc.tensor.matmul(
                        ps[:], xt[c][:, 128 * tt:128 * (tt + 1)], w_v[c][:],
                        start=(c == 0), stop=(c == 1))
                nc.vector.tensor_add(
                    vsb[:, tt, :, 0:HD],
                    ps[:].rearrange("p (h d) -> p h d", h=H),
                    b_v[:].rearrange("p (h d) -> p h d", h=H))

            stage_b(0, ms=(1, 3))
            for n in range(1, NQT):
                stage_b(n)

            # ---- stage D: attention, qt outer / head-pair, head inner ----
            # yt holds UNNORMALIZED out2T; sumexp rows are collected at
            # 32-partition spacing, normalized in one batched reciprocal per
            # q-tile, broadcast via K=1 matmuls, and multiplied in afterwards.
            yt = [ytp.tile([128, T], BF16, tag=f"yt{hp}", name=f"yt{hp}") for hp in range(2)]
            def norm_hp(qt, se, hp):
                # per-head-pair normalization: reciprocal of the two sumexp
                # rows, DRAM-bounce broadcast, one GpSimd multiply.
                rec_f = small.tile([97, QT], F32, tag=f"rec_f{hp}")
                p = 64 * hp
                nc.vector.reciprocal_approx_fast(rec_f[:], se[:])
                rdt = recdr.tile([2, QT], F32, tag=f"rd{hp}")
                for h in range(2):
                    nc.sync.dma_start(rdt[h:h + 1, :],
                                      rec_f[p + 32 * h:p + 32 * h + 1, :])
                bcS = small.tile([128, QT], F32, tag=f"bc{hp}")
                a = rdt[:]
                src_bc = bass.AP(tensor=a.tensor, offset=a.offset,
                                 ap=[[QT, 2], [0, 64], [1, QT]])
                nc.sync.dma_start(bcS[:], src_bc)
                ys = yt[hp][:, QT * qt:QT * (qt + 1)]
                nc.gpsimd.tensor_mul(ys, ys, bcS[:])

            def proj(qt):
                for tt in range(qt * QT // 128, (qt + 1) * QT // 128):
                    ps = scps.tile([128, C], F32, tag="sc", name="pps")
                    for c in range(2):
                        nc.tensor.matmul(
                            ps[:], yt[c][:, 128 * tt:128 * (tt + 1)], w_p[c][:],
                            start=(c == 0), stop=(c == 1))
                    ost = ostage.tile([128, C], F32, tag="ost")
                    nc.vector.tensor_add(ost[:], ps[:], b_p[:])
                    nc.sync.dma_start(out_ap[128 * tt:128 * (tt + 1), :], ost[:])

            PVLAG = 1
            pending = []   # deferred closures, injected at g==2 of later pairs
            for qt in range(NQT):
                se = small.tile([97, QT], F32, tag="se")
                for hp in range(2):
                    qT = qkt[hp]
                    kT = qkt[hp + 2]
                    o2 = [o2ps.tile([HD + 4, QT], F32, tag=f"o2{h}", name=f"o2{h}") for h in range(2)]
                    ex = [[None] * NG for _ in range(2)]
                    def pv(h, g):
                        nc.tensor.matmul(
                            o2[h][:],
                            vsb[:, 2 * g:2 * (g + 1), 2 * hp + h, :],
                            ex[h][g].rearrange("p (j n) -> p j n", j=2),
                            start=(g == 0), stop=(g == NG - 1),
                            perf_mode=mybir.MatmulPerfMode.DoubleRow)
                    for g in range(NG):
                        scs = []
                        for h in range(2):
                            sc = scps.tile([128, 2 * QT], F32, tag="sc")
                            for j in range(2):
                                i = 2 * g + j
                                nc.tensor.matmul(
                                    sc[:, QT * j:QT * (j + 1)],
                                    kT[64 * h:64 * (h + 1), 128 * i:128 * (i + 1)],
                                    qT[64 * h:64 * (h + 1), QT * qt:QT * (qt + 1)],
                                    start=True, stop=True)
                            scs.append(sc)
                        for h in range(2):
                            # exp split: h=0 on ScalarE (native exp -> fp8),
                            # h=1 on DVE (Schraudolph int8 -> fp8 bitcast).
                            if h == 1:
                                exi = expp.tile([128, 2 * QT], I8, tag="exv")
                                nc.vector.tensor_scalar(
                                    exi[:], scs[h][:], SCHRAUDOLPH_MUL,
                                    SCHRAUDOLPH_ADD, MULT, ADD)
                                ex[h][g] = exi[:].bitcast(F8)
                            else:
                                exf = expp.tile([128, 2 * QT], F8, tag="exs")
                                nc.scalar.activation(
                                    exf[:], scs[h][:],
                                    mybir.ActivationFunctionType.Exp,
                                    bias=0.0, scale=float(HD) ** -0.5)
                                ex[h][g] = exf[:]
                        if g >= PVLAG:
                            for h in range(2):
                                pv(h, g - PVLAG)
                        if g == 2 and pending:
                            pending.pop(0)()
                    for g in range(NG - PVLAG, NG):
                        for h in range(2):
                            pv(h, g)
                    for h in range(2):
                        habs = 2 * hp + h
                        nc.scalar.activation(
                            yt[hp][64 * h:64 * (h + 1), QT * qt:QT * (qt + 1)],
                            o2[h][0:HD, :],
                            mybir.ActivationFunctionType.Copy)
                        nc.scalar.activation(
                            se[32 * habs:32 * habs + 1, :],
                            o2[h][HD:HD + 1, :],
                            mybir.ActivationFunctionType.Copy)
                    pending.append(
                        lambda qt=qt, se=se, hp=hp: norm_hp(qt, se, hp))
                pending.append(lambda qt=qt: proj(qt))
            for fn in pending:
                fn()
    nc.compile()
    return nc


_NC_CACHE = []


def _get_nc():
    if not _NC_CACHE:
        _NC_CACHE.append(build_nc())
    return _NC_CACHE[0]


def make_in_maps(x, w_qkv, b_qkv, w_proj, b_proj):
    shared = {
        "w_qkT": np.ascontiguousarray(w_qkv[:2 * C].T, dtype=np.float32),
        "w_vT": np.ascontiguousarray(w_qkv[2 * C:].T, dtype=np.float32),
        "w_pT": np.ascontiguousarray(w_proj.T, dtype=np.float32),
        "b_qk": np.ascontiguousarray(b_qkv[:2 * C].reshape(4, 128), dtype=np.float32),
        "b_v": np.ascontiguousarray(b_qkv[2 * C:], dtype=np.float32),
        "b_p": np.ascontiguousarray(b_proj, dtype=np.float32),
    }
    return [dict(shared, x=np.ascontiguousarray(x[b], dtype=np.float32))
            for b in range(B)]


def run(x, w_qkv, b_qkv, w_proj, b_proj, trace=False):
    nc = _get_nc()
    in_maps = make_in_maps(np.asarray(x), np.asarray(w_qkv), np.asarray(b_qkv),
                           np.asarray(w_proj), np.asarray(b_proj))
    res = run_bass_kernel_spmd(nc, in_maps, list(range(N_CORES)), trace=trace)
    out = np.stack([res.results[b]["out"] for b in range(B)])
    return out, res


def kernel(x, w_qkv, b_qkv, w_proj, b_proj):
    out, _ = run(x, w_qkv, b_qkv, w_proj, b_proj, trace=False)
    return out
